# revision 21
# baseline (speedup 1.0000x reference)
"""Multi-head attention (B=2, S=2048, EMB=1024, H=16) on 8 Trainium2 cores.

Sharding: the 4096 (batch, seq) query tokens are split into 8 chunks of 512;
core c handles batch b = c // 4, query rows [512*(c%4), 512*(c%4+1)).  Each
core computes the K/V projections for its full batch (4x redundant, no
collectives), then attention over all 16 heads for its 512 queries, then the
output projection, writing a disjoint [512, 1024] slice of the output.

Layouts: all activations flow feature-major ("transposed", [emb, token]) and
weights are staged pre-transposed ([e_in, e_out]), pre-cast to bf16, and
pre-tiled to [128, e_in/128, n] on the host so every device DMA is a straight
contiguous copy.  The device does zero transposes:
  - qh/kh projections produce qh_T/kh_T [d, tok]   (lhsT = W.T, rhs = x.T)
  - vh projection produces vh [tok, d]             (lhsT = x.T, rhs = W.T)
  - scores_T [k, q] = kh_T.T @ qh_T                (softmax along partitions)
  - exp via ACT with scale=1/8, no max subtraction (|scores| <~ 7 so exp is
    safely in range; matches reference softmax up to rounding)
  - att_u_T [d+1, q] = vhe.T @ exp_T where vhe has an all-ones 65th column,
    so row 64 accumulates the softmax denominator for free
  - normalize: r = 1/denom (DVE), broadcast across partitions via a
    0-stride DMA, multiply into attT (DVE); deferred one head-pair so the
    reciprocal latency never stalls the in-order PE queue
  - out [q, e_out] = att_T.T @ Wo.T                (token-major, DMA-ready)

Head-dim is 64, so score matmuls are packed two heads per PE pass using
row-tiling (tile_position auto-derived from partition bases 0 / 64).  The
kh projection is interleaved into the attention pair loop: it is pure PE
work with no ACT dependency, which keeps the PE dense (and the HAM clock
warm) while the scalar engine chews through the exp() stream.
"""

import numpy as np
import ml_dtypes

import concourse.bass as bass  # noqa: F401
import concourse.mybir as mybir
import concourse.tile as tile
from concourse import bacc
from concourse.bass_utils import run_bass_kernel_spmd

BF = mybir.dt.bfloat16
F32 = mybir.dt.float32

EMB = 1024
HEADS = 16
HD = EMB // HEADS          # 64
B, S = 2, 2048
N_CORES = 8
QS = (B * S) // N_CORES    # 512 queries per core
P = 128
NE = EMB // P              # 8 emb chunks
NJ = S // P                # 16 key-token chunks
HPAIRS = HEADS // 2        # 8
EXPF = mybir.ActivationFunctionType.Exp
SCALE = 1.0 / np.sqrt(HD)  # 0.125


def _build_nc(with_bv: bool, with_bo: bool):
    from contextlib import ExitStack

    nc = bacc.Bacc(num_devices=N_CORES)
    dp = nc.declare_dram_parameter
    # activations / weights pre-tiled on host: [128, n_chunks, tokens/features]
    qT = dp("qT", [P, NE, QS], BF, isOutput=False)
    kT = dp("kT", [P, NE, S], BF, isOutput=False)
    vT = dp("vT", [P, NE, S], BF, isOutput=False)
    WqT = dp("WqT", [P, NE, EMB], BF, isOutput=False)
    WkT = dp("WkT", [P, NE, EMB], BF, isOutput=False)
    WvT = dp("WvT", [P, NE, EMB], BF, isOutput=False)
    WoT = dp("WoT", [P, NE, EMB], BF, isOutput=False)
    bqp = dp("bqp", [P, NE], F32, isOutput=False)
    bkp = dp("bkp", [P, NE], F32, isOutput=False)
    bvr = dp("bvr", [1, EMB], BF, isOutput=False)
    bor = dp("bor", [1, EMB], BF, isOutput=False)
    out = dp("out", [QS, EMB], F32, isOutput=True)

    with tile.TileContext(nc) as tc, ExitStack() as ctx:
        wpool = ctx.enter_context(tc.tile_pool(name="wts", bufs=1))
        apool = ctx.enter_context(tc.tile_pool(name="acts", bufs=1))

        # persistent tiles
        khT_sb = apool.tile([P, NE, S], BF, tag="khT")
        vhe_sb = apool.tile([P, NJ, HEADS, HD + 1], BF, tag="vhe")
        qhT_sb = apool.tile([P, NE, QS], BF, tag="qhT")
        attT_t = [
            apool.tile([P, QS], BF, tag=f"attT{i}", name=f"attT{i}")
            for i in range(HPAIRS)
        ]

        bqp_sb = wpool.tile([P, NE], F32, tag="bqp")
        nc.sync.dma_start(bqp_sb[:], bqp[:])
        bkp_sb = wpool.tile([P, NE], F32, tag="bkp")
        nc.sync.dma_start(bkp_sb[:], bkp[:])
        if with_bv or with_bo:
            ones_b = wpool.tile([1, P], BF, tag="onesb")
            nc.vector.memset(ones_b[:], 1.0)
        if with_bv:
            bvr_sb = wpool.tile([1, EMB], BF, tag="bvr")
            nc.sync.dma_start(bvr_sb[:], bvr[:])
        if with_bo:
            bor_sb = wpool.tile([1, EMB], BF, tag="bor")
            nc.sync.dma_start(bor_sb[:], bor[:])

        # ones column of vhe (denominator accumulator)
        nc.vector.memset(vhe_sb[:, :, :, HD], 1.0)

        # ---------------- qh + vh projections ----------------
        with (
            tc.tile_pool(name="wqv", bufs=1) as wqv,
            tc.tile_pool(name="vin", bufs=2) as vin,
            tc.tile_pool(name="pps", bufs=4, space="PSUM") as pps,
        ):
            # qh first: small chunked loads so the PE starts almost at once
            qT_sb = wqv.tile([P, NE, QS], BF, tag="qTb")
            WqT_sb = wqv.tile([P, NE, EMB], BF, tag="WqT")
            for kk in range(NE):
                nc.sync.dma_start(qT_sb[:, kk, :], qT[:, kk, :])
                nc.sync.dma_start(WqT_sb[:, kk, :], WqT[:, kk, :])
            WvT_sb = wqv.tile([P, NE, EMB], BF, tag="WvT")
            nc.sync.dma_start(WvT_sb[:], WvT[:])

            for mm in range(NE):
                ps = pps.tile([P, QS], F32, tag="pps")
                for kk in range(NE):
                    nc.tensor.matmul(
                        ps[:],
                        WqT_sb[:, kk, mm * P : (mm + 1) * P],
                        qT_sb[:, kk, :],
                        start=(kk == 0),
                        stop=(kk == NE - 1),
                    )
                nc.vector.tensor_scalar_add(
                    qhT_sb[:, mm, :], ps[:], bqp_sb[:, mm : mm + 1]
                )

            # vh [tok, emb_out] -> vhe (ones column preserved)
            for mo in range(S // QS):
                vT_blk = vin.tile([P, NE, QS], BF, tag="vTb")
                nc.sync.dma_start(vT_blk[:], vT[:, :, mo * QS : (mo + 1) * QS])
                for mi in range(QS // P):
                    mm = mo * (QS // P) + mi
                    for nn in range(EMB // QS):
                        ps = pps.tile([P, QS], F32, tag="pps")
                        for kk in range(NE):
                            nc.tensor.matmul(
                                ps[:],
                                vT_blk[:, kk, mi * P : (mi + 1) * P],
                                WvT_sb[:, kk, nn * QS : (nn + 1) * QS],
                                start=(kk == 0),
                                stop=(kk == NE - 1) and not with_bv,
                            )
                        if with_bv:
                            nc.tensor.matmul(
                                ps[:],
                                ones_b[:, :],
                                bvr_sb[:, nn * QS : (nn + 1) * QS],
                                start=False,
                                stop=True,
                            )
                        nc.vector.tensor_copy(
                            vhe_sb[:, mm, nn * 8 : (nn + 1) * 8, 0:HD],
                            ps[:].rearrange("p (h d) -> p h d", d=HD),
                        )

        # kh inputs: only needed from the first attention pair on
        WkT_sb = wpool.tile([P, NE, EMB], BF, tag="WkT")
        nc.sync.dma_start(WkT_sb[:], WkT[:])
        kT_sb = wpool.tile([P, NE, S], BF, tag="kTb")
        nc.sync.dma_start(kT_sb[:], kT[:])
        # Wo: loaded during attention so the output projection never waits
        WoT_sb = wpool.tile([P, NE, EMB], BF, tag="WoT")
        nc.sync.dma_start(WoT_sb[:], WoT[:])

        # ---------------- attention (kh interleaved) ----------------
        with (
            tc.tile_pool(name="scps", bufs=2, space="PSUM") as scps,
            tc.tile_pool(name="attps", bufs=4, space="PSUM") as attps,
            tc.tile_pool(name="ppool", bufs=17) as ppool,
            tc.tile_pool(name="dpool", bufs=5) as dpool,
        ):

            def normalize_pair(hp, att0, att1):
                """attT = att_u * (1/denom); deferred one pair so the
                reciprocal never stalls the in-order PE queue."""
                for hh, att_ps in ((0, att0), (1, att1)):
                    r_t = dpool.tile([P, QS], F32, tag="nrm")
                    r = r_t[HD : HD + 1, :]
                    nc.vector.reciprocal(r, att_ps[HD : HD + 1, :])
                    # hop to lane 0: partition_broadcast replicates lane 0
                    r0 = dpool.tile([1, QS], F32, tag="nrm")
                    nc.sync.dma_start(r0[:], r)
                    rb_sb = dpool.tile([HD, QS], F32, tag="nrm")
                    nc.gpsimd.partition_broadcast(rb_sb[:], r0[:])
                    if hh == 0:
                        nc.vector.tensor_mul(
                            attT_t[hp][0:HD, :], att_ps[0:HD, :], rb_sb[:]
                        )
                    else:
                        t1 = dpool.tile([HD, QS], BF, tag="nrm")
                        nc.vector.tensor_mul(t1[:], att_ps[0:HD, :], rb_sb[:])
                        nc.sync.dma_start(attT_t[hp][HD:P, :], t1[:])

            def kh_proj(mm):
                for nn in range(S // QS):
                    ps = scps.tile([P, QS], F32, tag="sc")
                    for kk in range(NE):
                        nc.tensor.matmul(
                            ps[:],
                            WkT_sb[:, kk, mm * P : (mm + 1) * P],
                            kT_sb[:, kk, nn * QS : (nn + 1) * QS],
                            start=(kk == 0),
                            stop=(kk == NE - 1),
                        )
                    nc.vector.tensor_scalar_add(
                        khT_sb[:, mm, nn * QS : (nn + 1) * QS],
                        ps[:],
                        bkp_sb[:, mm : mm + 1],
                    )

            def att_pair(php, patt0, patt1, pprobs):
                for jj in range(NJ // 2):
                    pp0, pp1 = pprobs[jj]
                    for t in range(2):
                        j = 2 * jj + t
                        first = jj == 0 and t == 0
                        last = jj == NJ // 2 - 1 and t == 1
                        nc.tensor.matmul(
                            patt0[:],
                            vhe_sb[:, j, 2 * php, :],
                            pp0[:, t * QS : (t + 1) * QS],
                            start=first,
                            stop=last,
                        )
                        nc.tensor.matmul(
                            patt1[:],
                            vhe_sb[:, j, 2 * php + 1, :],
                            pp1[:, t * QS : (t + 1) * QS],
                            start=first,
                            stop=last,
                        )

            def att_chunk(php, patt0, patt1, pprobs, jj):
                pp0, pp1 = pprobs[jj]
                for t in range(2):
                    j = 2 * jj + t
                    first = jj == 0 and t == 0
                    last = jj == NJ // 2 - 1 and t == 1
                    nc.tensor.matmul(
                        patt0[:],
                        vhe_sb[:, j, 2 * php, :],
                        pp0[:, t * QS : (t + 1) * QS],
                        start=first,
                        stop=last,
                    )
                    nc.tensor.matmul(
                        patt1[:],
                        vhe_sb[:, j, 2 * php + 1, :],
                        pp1[:, t * QS : (t + 1) * QS],
                        start=first,
                        stop=last,
                    )

            def sc_chunk(hp, jj):
                sc0 = scps.tile([P, 2 * QS], F32, tag="sc", name=f"sc0_{hp}_{jj}")
                sc1 = scps.tile([P, 2 * QS], F32, tag="sc", name=f"sc1_{hp}_{jj}")
                for t in range(2):
                    j = 2 * jj + t
                    nc.tensor.matmul(
                        sc0[:, t * QS : (t + 1) * QS],
                        khT_sb[0:HD, hp, j * P : (j + 1) * P],
                        qhT_sb[0:HD, hp, :],
                        start=True,
                        stop=True,
                    )
                    nc.tensor.matmul(
                        sc1[:, t * QS : (t + 1) * QS],
                        khT_sb[HD:P, hp, j * P : (j + 1) * P],
                        qhT_sb[HD:P, hp, :],
                        start=True,
                        stop=True,
                    )
                p0 = ppool.tile([P, 2 * QS], BF, tag="probs", name=f"p0_{hp}_{jj}")
                nc.scalar.activation(p0[:], sc0[:], EXPF, scale=SCALE)
                p1 = ppool.tile([P, 2 * QS], BF, tag="probs", name=f"p1_{hp}_{jj}")
                nc.scalar.activation(p1[:], sc1[:], EXPF, scale=SCALE)
                return (p0, p1)

            # software pipeline: pair hp computes its scores interleaved with
            # the previous pair's att matmuls so the in-order PE queue always
            # has non-score work while ACT drains the score PSUM slots; the
            # kh projection of the NEXT pair fills the PE while ACT finishes.
            kh_proj(0)
            pend = None  # (hp, att0, att1, probs)
            for hp in range(HPAIRS):
                att0 = attps.tile([HD + 1, QS], F32, tag="att", name=f"att0_{hp}")
                att1 = attps.tile([HD + 1, QS], F32, tag="att", name=f"att1_{hp}")
                probs = []
                for jj in range(NJ // 2):
                    probs.append(sc_chunk(hp, jj))
                    if pend is not None:
                        att_chunk(pend[0], pend[1], pend[2], pend[3], jj)
                if pend is not None:
                    normalize_pair(*pend[:3])
                if hp + 1 < HPAIRS:
                    kh_proj(hp + 1)
                pend = (hp, att0, att1, probs)

            # drain: last pair's att matmuls + normalize
            for jj in range(NJ // 2):
                att_chunk(pend[0], pend[1], pend[2], pend[3], jj)
            normalize_pair(*pend[:3])

        # ---------------- output projection ----------------
        with (
            tc.tile_pool(name="ops", bufs=2, space="PSUM") as ops,
            tc.tile_pool(name="osb", bufs=3) as osb,
        ):
            for mq in range(QS // P):
                for nn in range(EMB // QS):
                    ps = ops.tile([P, QS], F32, tag="ops")
                    for hp in range(HPAIRS):
                        nc.tensor.matmul(
                            ps[:],
                            attT_t[hp][:, mq * P : (mq + 1) * P],
                            WoT_sb[:, hp, nn * QS : (nn + 1) * QS],
                            start=(hp == 0),
                            stop=(hp == HPAIRS - 1) and not with_bo,
                        )
                    if with_bo:
                        nc.tensor.matmul(
                            ps[:],
                            ones_b[:, :],
                            bor_sb[:, nn * QS : (nn + 1) * QS],
                            start=False,
                            stop=True,
                        )
                    ob = osb.tile([P, QS], F32, tag="ob")
                    nc.vector.tensor_copy(ob[:], ps[:])
                    nc.sync.dma_start(
                        out[mq * P : (mq + 1) * P, nn * QS : (nn + 1) * QS], ob[:]
                    )

    nc.finalize()
    return nc


_NC_CACHE: dict = {}


def _get_nc(with_bv: bool, with_bo: bool):
    key = (with_bv, with_bo)
    if key not in _NC_CACHE:
        _NC_CACHE[key] = _build_nc(*key)
    return _NC_CACHE[key]


def _feat_tiled(xT):
    """[EMB, n] -> [128, NE, n] contiguous (feature chunks on partitions)."""
    n = xT.shape[1]
    return np.ascontiguousarray(xT.reshape(NE, P, n).transpose(1, 0, 2))


def _stage(inputs):
    bf = ml_dtypes.bfloat16
    f32 = np.float32

    def arr(name):
        return np.asarray(inputs[name], f32)

    q, k, v = arr("q"), arr("k"), arr("v")
    Wq, Wk, Wv, Wo = arr("Wq"), arr("Wk"), arr("Wv"), arr("Wo")
    bq, bk, bv, bo = arr("bq"), arr("bk"), arr("bv"), arr("bo")

    with_bv = bool(np.any(bv))
    with_bo = bool(np.any(bo))

    def wt(W):  # W.T tiled: [128, NE, EMB] bf16
        return _feat_tiled(np.ascontiguousarray(W.T)).astype(bf)

    def xt(x2d):  # x.T tiled: [128, NE, tokens] bf16
        return _feat_tiled(np.ascontiguousarray(x2d.T)).astype(bf)

    common = {
        "WqT": wt(Wq),
        "WkT": wt(Wk),
        "WvT": wt(Wv),
        "WoT": wt(Wo),
        "bqp": np.ascontiguousarray(bq.reshape(NE, P).T),
        "bkp": np.ascontiguousarray(bk.reshape(NE, P).T),
        "bvr": bv.reshape(1, EMB).astype(bf),
        "bor": bo.reshape(1, EMB).astype(bf),
    }
    kT_b = [xt(k[b_]) for b_ in range(B)]
    vT_b = [xt(v[b_]) for b_ in range(B)]

    in_maps = []
    for c in range(N_CORES):
        b_, g = divmod(c, N_CORES // B)
        m = dict(common)
        m["qT"] = xt(q[b_, g * QS : (g + 1) * QS, :])
        m["kT"] = kT_b[b_]
        m["vT"] = vT_b[b_]
        in_maps.append(m)
    return in_maps, with_bv, with_bo


def _assemble(results):
    out = np.empty((B, S, EMB), np.float32)
    for c in range(N_CORES):
        b_, g = divmod(c, N_CORES // B)
        out[b_, g * QS : (g + 1) * QS, :] = results[c]["out"]
    return out


def kernel(**inputs) -> np.ndarray:
    in_maps, with_bv, with_bo = _stage(inputs)
    nc = _get_nc(with_bv, with_bo)
    res = run_bass_kernel_spmd(nc, in_maps, list(range(N_CORES)))
    return _assemble(res.results)
